# revision 44
# baseline (speedup 1.0000x reference)
"""Trainium2 Bass kernel for nn_NumAttention (sparse_attention).

Reference computation (per batch b, head i):
    k     = blockmix(x_cat, softmax(W_K)[i])            # [P, DH]
    xq    = blockmix(x_cat, softmax(W_Q)[i])            # [P, DH]
    q     = xq @ softmax(W_pred)[i]                     # [P, DH]
    v     = x_num @ softmax(W_V)[i]                     # [P]
    z[qp] = sum_{p<=qp} v[p] * (k[p] . q[qp])           # causal, no softmax

Key restructuring: attention here is softmax-free with scalar values, so it
is *linear*:  z[qp] = xq[qp] . S[qp]  with  S = cumsum_p(v[p] * ktilde[p,:])
where ktilde = k @ pp^T folds the W_pred mix into the k side.  The O(P^2)
score matrix is never materialized; per-core device work is one
[P,512]x[512,256] bf16 mix matmul (fp32 accumulate) plus a chunked cumsum.

The chunked cumsum: per 128-position chunk, S_c = triT_incl @ vk_c with the
inter-chunk carry folded in by *adding the exclusive block prefix Tex[c] to
vk_c's first row* - the inclusive triangular matmul then propagates it to
every row of the chunk.  This keeps pass 2 at one bf16 matmul per chunk
pair with a single stationary operand (no LDWEIGHTS churn, no carry
matmuls).  Block sums ride one accumulating PSUM tile; the 8-row prefix is
three tiny matmuls.

Sharding: 8 cores = 4 batches x 2 head-groups (4 heads each).  Host ships
x_cat[b] pre-transposed to feature-major bf16 (halves HBM traffic, no
on-device transposes), the tiny per-head effective weight matrices, and
host-computed v (x_num @ pv^T, 8 MFLOP).  A short burst of dummy matmuls
during the DMA head warms the PE HAM clock gate before the real mix stream.
"""

import numpy as np
import ml_dtypes

import concourse.bacc as bacc
import concourse.mybir as mybir
import concourse.tile as tile
from concourse.tile import add_dep_helper
from concourse.bass_utils import run_bass_kernel_spmd

B, P, DC, DN, H, DH = 4, 2048, 512, 64, 8, 64
NV = DC // DH
CH = 128          # positions per chunk
NCH = P // CH     # 16 chunks
NPR = NCH // 2    # 8 chunk pairs
HPC = 4           # heads per core
FH = HPC * DH     # 256 = stacked-head free width
FH2 = 2 * FH      # 512 = pair width
NCORES = 8
KC = DC // CH     # 4 feature K-chunks
NWARM = 12        # PE warm-up dummy matmuls

_BF16 = ml_dtypes.bfloat16

_cache = {}


def _softmax(x, axis=-1):
    e = np.exp(x - x.max(axis=axis, keepdims=True))
    return e / e.sum(axis=axis, keepdims=True)


def _build_program():
    nc = bacc.Bacc()
    f32 = mybir.dt.float32
    bf16 = mybir.dt.bfloat16
    mult = mybir.AluOpType.mult
    add = mybir.AluOpType.add

    # x_cat[b] transposed + quarter-blocked on host, packed with W into
    # two big per-partition-contiguous transfers: bigA = [W, xq0, xq1]
    # (12KB/partition) gates the mix start; bigB = [xq2, xq3] follows.
    biga_d = nc.dram_tensor("biga", [CH, 3 * KC, FH2], bf16, kind="ExternalInput")
    bigb_d = nc.dram_tensor("bigb", [CH, 2 * KC, FH2], bf16, kind="ExternalInput")
    # host-computed v in pos-chunk-major layout [p, (chunk, head)]
    v_d = nc.dram_tensor("v", [CH, NCH * HPC], f32, kind="ExternalInput")
    trit_d = nc.dram_tensor("trit", [CH, CH], bf16, kind="ExternalInput")
    # oneh[:, j*NPR + m] = (m == j): pair-j chunk-sum selector columns
    oneh_d = nc.dram_tensor("oneh", [CH, NPR * NPR], bf16, kind="ExternalInput")
    strt_d = nc.dram_tensor("strt", [NPR, NPR], bf16, kind="ExternalInput")
    eye8_d = nc.dram_tensor("eye8", [NPR, NPR], bf16, kind="ExternalInput")
    # sel[k, j*128+p] = (k == j): carry-broadcast selector
    sel_d = nc.dram_tensor("sel", [NPR, NPR * CH], bf16, kind="ExternalInput")
    z_d = nc.dram_tensor("z", [CH, NCH * HPC], f32, kind="ExternalOutput")

    with tile.TileContext(nc) as tc:
        with (
            tc.tile_pool(name="persist", bufs=1) as pers,
            tc.tile_pool(name="work", bufs=3) as work,
            tc.tile_pool(name="mixp", bufs=3, space="PSUM") as mixp,
            tc.tile_pool(name="sp", bufs=2, space="PSUM") as sp,
            tc.tile_pool(name="psmall", bufs=1, space="PSUM") as psmall,
        ):
            biga_sb = pers.tile([CH, 3 * KC, FH2], bf16, tag="biga_sb")
            bigb_sb = pers.tile([CH, 2 * KC, FH2], bf16, tag="bigb_sb")
            v_sb = pers.tile([CH, NCH * HPC], f32, tag="v_sb")
            trit_sb = pers.tile([CH, CH], bf16, tag="trit_sb")
            oneh_sb = pers.tile([CH, NPR * NPR], bf16, tag="oneh_sb")
            strt_sb = pers.tile([NPR, NPR], bf16, tag="strt_sb")
            eye8_sb = pers.tile([NPR, NPR], bf16, tag="eye8_sb")
            sel_sb = pers.tile([NPR, NPR * CH], bf16, tag="sel_sb")
            vk_sb = pers.tile([CH, NCH, FH], bf16, tag="vk_sb")
            q_sb = pers.tile([CH, NCH, FH], bf16, tag="q_sb")
            t2_sb = pers.tile([NPR, FH2], bf16, tag="t2_sb")
            tsum_sb = pers.tile([NPR, FH], bf16, tag="tsum_sb")
            texw_sb = pers.tile([NPR, FH2], bf16, tag="texw_sb")
            z_sb = pers.tile([CH, NCH * HPC], f32, tag="z_sb")
            dumw = pers.tile([CH, FH2], bf16, tag="dumw")

            # ---- PE warm-up: dummy matmuls on a memset tile release the HAM
            # clock throttle while the xcT DMA is still in flight
            nc.gpsimd.memset(dumw[:], 0.0)
            psum_dum = psmall.tile([CH, FH2], f32, tag="psum_dum")
            for i in range(NWARM):
                nc.tensor.matmul(
                    psum_dum[:], dumw[:, 0:CH], dumw[:], start=True, stop=True
                )

            # ---- loads (all plain HWDGE; x_cat pre-transposed on host).
            # The SDMA engines drain ALL outstanding transfers round-robin at
            # packet granularity, so an unordered burst makes the first xcT
            # slice (which gates the whole mix stream) finish last.  Stage the
            # transfers with explicit sync deps: {w, v, s0} first, then xcT
            # slice pairs in consumption order, constants last.
            dma_a = nc.sync.dma_start(out=biga_sb[:], in_=biga_d[:])
            late_dmas = [
                nc.sync.dma_start(out=bigb_sb[:], in_=bigb_d[:]),
                nc.scalar.dma_start(out=v_sb[:], in_=v_d[:]),
                nc.scalar.dma_start(out=trit_sb[:], in_=trit_d[:]),
                nc.scalar.dma_start(out=oneh_sb[:], in_=oneh_d[:]),
                nc.scalar.dma_start(out=strt_sb[:], in_=strt_d[:]),
                nc.scalar.dma_start(out=eye8_sb[:], in_=eye8_d[:]),
                nc.scalar.dma_start(out=sel_sb[:], in_=sel_d[:]),
            ]
            # bigA must own the full DMA bandwidth (SDMA engines round-robin
            # every outstanding transfer, which would delay the mix start)
            for d in late_dmas:
                add_dep_helper(
                    d.ins, dma_a.ins, sync=True, reason="tail loads after bigA"
                )

            # ---- pass 1: mix matmuls -> vk (bf16), q (bf16)
            for c in range(NCH):
                psum_mix = mixp.tile([CH, FH2], f32, tag="psum_mix")
                q = c // 4
                xq = biga_sb if q < 2 else bigb_sb
                xbase = (q + 1) * KC if q < 2 else (q - 2) * KC
                for kc in range(KC):
                    nc.tensor.matmul(
                        psum_mix[:],
                        xq[:, xbase + kc, (c % 4) * CH : (c % 4 + 1) * CH],
                        biga_sb[:, kc, :],
                        start=(kc == 0),
                        stop=(kc == KC - 1),
                    )
                # vk[p, i, h] = ktilde[p, i, h] * v[p, i]
                nc.vector.tensor_tensor(
                    out=vk_sb[:, c, :].rearrange("p (i h) -> p i h", h=DH),
                    in0=psum_mix[:, 0:FH].rearrange("p (i h) -> p i h", h=DH),
                    in1=v_sb[:, c * HPC : (c + 1) * HPC].unsqueeze(2).broadcast_to(
                        [CH, HPC, DH]
                    ),
                    op=mult,
                )
                nc.scalar.copy(q_sb[:, c, :], psum_mix[:, FH:FH2])

            # ---- paired chunk sums (after the mix stream so the in-order PE
            # never stalls on DVE vk completion mid-mix)
            psum_T2 = psmall.tile([NPR, FH2], f32, tag="psum_T2")
            for j in range(NPR):
                # T2[j] = [colsum(vk_{2j}) | colsum(vk_{2j+1})]
                nc.tensor.matmul(
                    psum_T2[:],
                    oneh_sb[:, j * NPR : (j + 1) * NPR],
                    vk_sb[:, 2 * j : 2 * j + 2, :].rearrange("p c f -> p (c f)"),
                    start=(j == 0),
                    stop=(j == NPR - 1),
                )

            # ---- block prefix on pair sums.  Dummy matmuls keep the PE (and
            # its HAM clock) busy while the short DVE/ACT prefix chain runs.
            for i in range(4):
                nc.tensor.matmul(
                    psum_dum[:], dumw[:, 0:CH], dumw[:], start=True, stop=True
                )
            nc.vector.tensor_copy(t2_sb[:], psum_T2[:])
            nc.vector.tensor_tensor(
                out=tsum_sb[:], in0=t2_sb[:, 0:FH], in1=t2_sb[:, FH:FH2], op=add
            )
            psum_texw = psmall.tile([NPR, FH2], f32, tag="psum_texw")
            # left half: Tex[2j]   = sum_{j'<j} Tsum[j']
            nc.tensor.matmul(
                psum_texw[:, 0:FH], strt_sb[:], tsum_sb[:], start=True, stop=True
            )
            # right half: Tex[2j+1] = Tex[2j] + T[2j]
            nc.tensor.matmul(
                psum_texw[:, FH:FH2], strt_sb[:], tsum_sb[:], start=True, stop=False
            )
            nc.tensor.matmul(
                psum_texw[:, FH:FH2], eye8_sb[:], t2_sb[:, 0:FH], start=False, stop=True
            )
            nc.vector.tensor_copy(texw_sb[:], psum_texw[:])
            for i in range(6):
                nc.tensor.matmul(
                    psum_dum[:], dumw[:, 0:CH], dumw[:], start=True, stop=True
                )

            # ---- pass 2 (paired): S = triT @ vk + carry;  z = rowsum(q * S)
            for j in range(NPR):
                psum_S = sp.tile([CH, FH2], f32, tag="psum_S")
                nc.tensor.matmul(
                    psum_S[:],
                    trit_sb[:],
                    vk_sb[:, 2 * j : 2 * j + 2, :].rearrange("p c f -> p (c f)"),
                    start=True,
                    stop=False,
                )
                nc.tensor.matmul(
                    psum_S[:],
                    sel_sb[:, j * CH : (j + 1) * CH],
                    texw_sb[:],
                    start=False,
                    stop=True,
                )
                # drain S to SBUF on the otherwise-idle ACT engine so both
                # DVE ops below run on 16-bit SBUF operands (fast path)
                s_sb = work.tile([CH, FH2], bf16, tag="s_sb")
                nc.scalar.copy(s_sb[:], psum_S[:])
                prod = work.tile([CH, FH2], bf16, tag="prod")
                nc.vector.tensor_tensor(
                    out=prod[:],
                    in0=q_sb[:, 2 * j : 2 * j + 2, :].rearrange("p c f -> p (c f)"),
                    in1=s_sb[:],
                    op=mult,
                )
                nc.vector.tensor_reduce(
                    out=z_sb[:, 2 * j * HPC : (2 * j + 2) * HPC].rearrange(
                        "p (c i) -> p c i", i=HPC
                    ),
                    in_=prod[:].rearrange("p (c i h) -> p c i h", i=HPC, h=DH),
                    axis=mybir.AxisListType.X,
                    op=add,
                )

            nc.sync.dma_start(out=z_d[:], in_=z_sb[:])

    nc.finalize()
    return nc


def _host_inputs(x_cat, x_num, W_K, W_Q, W_pred, W_V):
    """Per-core input maps. Core c = batch (c//2), head-group (c%2)."""
    pk = _softmax(W_K.astype(np.float64)).astype(np.float32)
    pq = _softmax(W_Q.astype(np.float64)).astype(np.float32)
    pp = _softmax(W_pred.astype(np.float64)).astype(np.float32)
    pv = _softmax(W_V.astype(np.float64)).astype(np.float32)

    trit = np.triu(np.ones((CH, CH), np.float32))
    oneh = np.zeros((CH, NPR, NPR), np.float32)
    oneh[:, np.arange(NPR), np.arange(NPR)] = 1.0
    oneh = oneh.reshape(CH, NPR * NPR)
    strt = np.triu(np.ones((NPR, NPR), np.float32), k=1)
    eye8 = np.eye(NPR, dtype=np.float32)
    sel = np.zeros((NPR, NPR, CH), np.float32)
    sel[np.arange(NPR), np.arange(NPR), :] = 1.0
    sel = sel.reshape(NPR, NPR * CH)

    eye = np.eye(DH, dtype=np.float32)
    v_full = np.einsum("bpd,id->bpi", x_num, pv)  # [B, P, H] fp32, host-side

    in_maps = []
    for core in range(NCORES):
        b, hg = core // 2, core % 2
        heads = range(hg * HPC, (hg + 1) * HPC)
        W = np.zeros((DC, FH2), np.float32)
        for j, i in enumerate(heads):
            # ktilde cols: W[(v,g), j*64+h] = pk[i,v] * pp[i,h,g]
            W[:, j * DH : (j + 1) * DH] = (
                pk[i][:, None, None] * pp[i].T[None, :, :]
            ).reshape(DC, DH)
            # xq cols: W[(v,h), FH + j*64+h'] = pq[i,v] * delta(h,h')
            W[:, FH + j * DH : FH + (j + 1) * DH] = np.kron(pq[i][:, None], eye)
        # pack [W, xq0, xq1] / [xq2, xq3]: per-partition contiguous blocks
        xq = x_cat[b].T.reshape(KC, CH, 4, P // 4).transpose(2, 1, 0, 3)  # [4,CH,KC,512]
        wq = W.reshape(KC, CH, FH2).transpose(1, 0, 2)                    # [CH,KC,512]
        biga = np.concatenate([wq, xq[0], xq[1]], axis=1)                 # [CH,12,512]
        bigb = np.concatenate([xq[2], xq[3]], axis=1)                     # [CH,8,512]
        # v in device layout [p, (chunk, head)]
        v_core = v_full[b][:, hg * HPC : (hg + 1) * HPC]  # [P, HPC]
        v_dev = np.ascontiguousarray(
            v_core.reshape(NCH, CH, HPC).transpose(1, 0, 2).reshape(CH, NCH * HPC)
        )
        in_maps.append(
            {
                "biga": biga.astype(_BF16),
                "bigb": bigb.astype(_BF16),
                "v": v_dev,
                "trit": trit.astype(_BF16),
                "oneh": oneh.astype(_BF16),
                "strt": strt.astype(_BF16),
                "eye8": eye8.astype(_BF16),
                "sel": sel.astype(_BF16),
            }
        )
    return in_maps


def _run(inputs, **spmd_kwargs):
    if "nc" not in _cache:
        _cache["nc"] = _build_program()
    nc = _cache["nc"]

    in_maps = _host_inputs(**inputs)
    res = run_bass_kernel_spmd(nc, in_maps, list(range(NCORES)), **spmd_kwargs)

    out = np.zeros((B, P, H), np.float32)
    for core in range(NCORES):
        b, hg = core // 2, core % 2
        z = res.results[core]["z"]  # [128, NCH*HPC]
        z = z.reshape(CH, NCH, HPC).transpose(1, 0, 2).reshape(P, HPC)
        out[b, :, hg * HPC : (hg + 1) * HPC] = z
    return out, res


def kernel(x_cat, x_num, W_K, W_Q, W_pred, W_V):
    out, _ = _run(
        dict(x_cat=x_cat, x_num=x_num, W_K=W_K, W_Q=W_Q, W_pred=W_pred, W_V=W_V)
    )
    return out


# revision 46
# speedup vs baseline: 1.0269x; 1.0269x over previous
"""Trainium2 Bass kernel for nn_NumAttention (sparse_attention).

Reference computation (per batch b, head i):
    k     = blockmix(x_cat, softmax(W_K)[i])            # [P, DH]
    xq    = blockmix(x_cat, softmax(W_Q)[i])            # [P, DH]
    q     = xq @ softmax(W_pred)[i]                     # [P, DH]
    v     = x_num @ softmax(W_V)[i]                     # [P]
    z[qp] = sum_{p<=qp} v[p] * (k[p] . q[qp])           # causal, no softmax

Key restructuring: attention here is softmax-free with scalar values, so it
is *linear*:  z[qp] = xq[qp] . S[qp]  with  S = cumsum_p(v[p] * ktilde[p,:])
where ktilde = k @ pp^T folds the W_pred mix into the k side.  The O(P^2)
score matrix is never materialized; per-core device work is one
[P,512]x[512,256] bf16 mix matmul (fp32 accumulate) plus a chunked cumsum.

The chunked cumsum: per 128-position chunk, S_c = triT_incl @ vk_c with the
inter-chunk carry folded in by *adding the exclusive block prefix Tex[c] to
vk_c's first row* - the inclusive triangular matmul then propagates it to
every row of the chunk.  This keeps pass 2 at one bf16 matmul per chunk
pair with a single stationary operand (no LDWEIGHTS churn, no carry
matmuls).  Block sums ride one accumulating PSUM tile; the 8-row prefix is
three tiny matmuls.

Sharding: 8 cores = 4 batches x 2 head-groups (4 heads each).  Host ships
x_cat[b] pre-transposed to feature-major bf16 (halves HBM traffic, no
on-device transposes), the tiny per-head effective weight matrices, and
host-computed v (x_num @ pv^T, 8 MFLOP).  A short burst of dummy matmuls
during the DMA head warms the PE HAM clock gate before the real mix stream.
"""

import numpy as np
import ml_dtypes

import concourse.bacc as bacc
import concourse.mybir as mybir
import concourse.tile as tile
from concourse.tile import add_dep_helper
from concourse.bass_utils import run_bass_kernel_spmd

B, P, DC, DN, H, DH = 4, 2048, 512, 64, 8, 64
NV = DC // DH
CH = 128          # positions per chunk
NCH = P // CH     # 16 chunks
NPR = NCH // 2    # 8 chunk pairs
HPC = 4           # heads per core
FH = HPC * DH     # 256 = stacked-head free width
FH2 = 2 * FH      # 512 = pair width
NCORES = 8
KC = DC // CH     # 4 feature K-chunks
NWARM = 10        # PE warm-up dummy matmuls

_BF16 = ml_dtypes.bfloat16

_cache = {}


def _softmax(x, axis=-1):
    e = np.exp(x - x.max(axis=axis, keepdims=True))
    return e / e.sum(axis=axis, keepdims=True)


def _build_program():
    nc = bacc.Bacc()
    f32 = mybir.dt.float32
    bf16 = mybir.dt.bfloat16
    mult = mybir.AluOpType.mult
    add = mybir.AluOpType.add

    # x_cat[b] transposed + eighth-blocked on host so each 256KB slice
    # DMA is per-partition contiguous (2KB descriptors)
    w_d = nc.dram_tensor("w", [CH, KC, FH2], bf16, kind="ExternalInput")
    xct_d = nc.dram_tensor("xct", [8, CH, KC, P // 8], bf16, kind="ExternalInput")
    # host-computed v in pos-chunk-major layout [p, (chunk, head)]
    v_d = nc.dram_tensor("v", [CH, NCH * HPC], f32, kind="ExternalInput")
    trit_d = nc.dram_tensor("trit", [CH, CH], bf16, kind="ExternalInput")
    # oneh[:, j*NPR + m] = (m == j): pair-j chunk-sum selector columns
    oneh_d = nc.dram_tensor("oneh", [CH, NPR * NPR], bf16, kind="ExternalInput")
    strt_d = nc.dram_tensor("strt", [NPR, NPR], bf16, kind="ExternalInput")
    eye8_d = nc.dram_tensor("eye8", [NPR, NPR], bf16, kind="ExternalInput")
    # sel[k, j*128+p] = (k == j): carry-broadcast selector
    sel_d = nc.dram_tensor("sel", [NPR, NPR * CH], bf16, kind="ExternalInput")
    z_d = nc.dram_tensor("z", [CH, NCH * HPC], f32, kind="ExternalOutput")

    with tile.TileContext(nc) as tc:
        with (
            tc.tile_pool(name="persist", bufs=1) as pers,
            tc.tile_pool(name="work", bufs=3) as work,
            tc.tile_pool(name="mixp", bufs=3, space="PSUM") as mixp,
            tc.tile_pool(name="sp", bufs=2, space="PSUM") as sp,
            tc.tile_pool(name="psmall", bufs=1, space="PSUM") as psmall,
        ):
            w_sb = pers.tile([CH, KC, FH2], bf16, tag="w_sb")
            xcT = pers.tile([CH, 8, KC, P // 8], bf16, tag="xcT")
            v_sb = pers.tile([CH, NCH * HPC], f32, tag="v_sb")
            trit_sb = pers.tile([CH, CH], bf16, tag="trit_sb")
            oneh_sb = pers.tile([CH, NPR * NPR], bf16, tag="oneh_sb")
            strt_sb = pers.tile([NPR, NPR], bf16, tag="strt_sb")
            eye8_sb = pers.tile([NPR, NPR], bf16, tag="eye8_sb")
            sel_sb = pers.tile([NPR, NPR * CH], bf16, tag="sel_sb")
            vk_sb = pers.tile([CH, NCH, FH], bf16, tag="vk_sb")
            q_sb = pers.tile([CH, NCH, FH], bf16, tag="q_sb")
            t2_sb = pers.tile([NPR, FH2], bf16, tag="t2_sb")
            tsum_sb = pers.tile([NPR, FH], bf16, tag="tsum_sb")
            texw_sb = pers.tile([NPR, FH2], bf16, tag="texw_sb")
            z_sb = pers.tile([CH, NCH * HPC], f32, tag="z_sb")
            dumw = pers.tile([CH, FH2], bf16, tag="dumw")

            # ---- PE warm-up: dummy matmuls on a memset tile release the HAM
            # clock throttle while the xcT DMA is still in flight
            nc.gpsimd.memset(dumw[:], 0.0)
            psum_dum = psmall.tile([CH, FH2], f32, tag="psum_dum")
            for i in range(NWARM):
                nc.tensor.matmul(
                    psum_dum[:], dumw[:, 0:CH], dumw[:], start=True, stop=True
                )

            # ---- loads (all plain HWDGE; x_cat pre-transposed on host).
            # The SDMA engines drain ALL outstanding transfers round-robin at
            # packet granularity, so an unordered burst makes the first xcT
            # slice (which gates the whole mix stream) finish last.  Stage the
            # transfers with explicit sync deps: {w, v, s0} first, then xcT
            # slice pairs in consumption order, constants last.
            for s in range(8):
                nc.sync.dma_start(out=xcT[:, s], in_=xct_d[s])
            nc.scalar.dma_start(out=w_sb[:], in_=w_d[:])
            nc.scalar.dma_start(out=v_sb[:], in_=v_d[:])
            nc.scalar.dma_start(out=trit_sb[:], in_=trit_d[:])
            nc.scalar.dma_start(out=oneh_sb[:], in_=oneh_d[:])
            nc.scalar.dma_start(out=strt_sb[:], in_=strt_d[:])
            nc.scalar.dma_start(out=eye8_sb[:], in_=eye8_d[:])
            nc.scalar.dma_start(out=sel_sb[:], in_=sel_d[:])

            # ---- pass 1: mix matmuls -> vk (bf16), q (bf16)
            for c in range(NCH):
                psum_mix = mixp.tile([CH, FH2], f32, tag="psum_mix")
                for kc in range(KC):
                    nc.tensor.matmul(
                        psum_mix[:],
                        xcT[:, c // 2, kc, (c % 2) * CH : (c % 2 + 1) * CH],
                        w_sb[:, kc, :],
                        start=(kc == 0),
                        stop=(kc == KC - 1),
                    )
                # vk[p, i, h] = ktilde[p, i, h] * v[p, i]
                nc.vector.tensor_tensor(
                    out=vk_sb[:, c, :].rearrange("p (i h) -> p i h", h=DH),
                    in0=psum_mix[:, 0:FH].rearrange("p (i h) -> p i h", h=DH),
                    in1=v_sb[:, c * HPC : (c + 1) * HPC].unsqueeze(2).broadcast_to(
                        [CH, HPC, DH]
                    ),
                    op=mult,
                )
                nc.scalar.copy(q_sb[:, c, :], psum_mix[:, FH:FH2])

            # ---- paired chunk sums (after the mix stream so the in-order PE
            # never stalls on DVE vk completion mid-mix)
            psum_T2 = psmall.tile([NPR, FH2], f32, tag="psum_T2")
            for j in range(NPR):
                # T2[j] = [colsum(vk_{2j}) | colsum(vk_{2j+1})]
                nc.tensor.matmul(
                    psum_T2[:],
                    oneh_sb[:, j * NPR : (j + 1) * NPR],
                    vk_sb[:, 2 * j : 2 * j + 2, :].rearrange("p c f -> p (c f)"),
                    start=(j == 0),
                    stop=(j == NPR - 1),
                )

            # ---- block prefix on pair sums.  Dummy matmuls keep the PE (and
            # its HAM clock) busy while the short DVE/ACT prefix chain runs.
            for i in range(4):
                nc.tensor.matmul(
                    psum_dum[:], dumw[:, 0:CH], dumw[:], start=True, stop=True
                )
            nc.vector.tensor_copy(t2_sb[:], psum_T2[:])
            nc.vector.tensor_tensor(
                out=tsum_sb[:], in0=t2_sb[:, 0:FH], in1=t2_sb[:, FH:FH2], op=add
            )
            psum_texw = psmall.tile([NPR, FH2], f32, tag="psum_texw")
            # left half: Tex[2j]   = sum_{j'<j} Tsum[j']
            nc.tensor.matmul(
                psum_texw[:, 0:FH], strt_sb[:], tsum_sb[:], start=True, stop=True
            )
            # right half: Tex[2j+1] = Tex[2j] + T[2j]
            nc.tensor.matmul(
                psum_texw[:, FH:FH2], strt_sb[:], tsum_sb[:], start=True, stop=False
            )
            nc.tensor.matmul(
                psum_texw[:, FH:FH2], eye8_sb[:], t2_sb[:, 0:FH], start=False, stop=True
            )
            nc.vector.tensor_copy(texw_sb[:], psum_texw[:])
            for i in range(6):
                nc.tensor.matmul(
                    psum_dum[:], dumw[:, 0:CH], dumw[:], start=True, stop=True
                )

            # ---- pass 2 (paired): S = triT @ vk + carry;  z = rowsum(q * S)
            for j in range(NPR):
                psum_S = sp.tile([CH, FH2], f32, tag="psum_S")
                nc.tensor.matmul(
                    psum_S[:],
                    trit_sb[:],
                    vk_sb[:, 2 * j : 2 * j + 2, :].rearrange("p c f -> p (c f)"),
                    start=True,
                    stop=False,
                )
                nc.tensor.matmul(
                    psum_S[:],
                    sel_sb[:, j * CH : (j + 1) * CH],
                    texw_sb[:],
                    start=False,
                    stop=True,
                )
                # drain S to SBUF on the otherwise-idle ACT engine so both
                # DVE ops below run on 16-bit SBUF operands (fast path)
                s_sb = work.tile([CH, FH2], bf16, tag="s_sb")
                nc.scalar.copy(s_sb[:], psum_S[:])
                prod = work.tile([CH, FH2], bf16, tag="prod")
                nc.vector.tensor_tensor(
                    out=prod[:],
                    in0=q_sb[:, 2 * j : 2 * j + 2, :].rearrange("p c f -> p (c f)"),
                    in1=s_sb[:],
                    op=mult,
                )
                nc.vector.tensor_reduce(
                    out=z_sb[:, 2 * j * HPC : (2 * j + 2) * HPC].rearrange(
                        "p (c i) -> p c i", i=HPC
                    ),
                    in_=prod[:].rearrange("p (c i h) -> p c i h", i=HPC, h=DH),
                    axis=mybir.AxisListType.X,
                    op=add,
                )

            nc.sync.dma_start(out=z_d[:], in_=z_sb[:])

    nc.finalize()
    return nc


def _host_inputs(x_cat, x_num, W_K, W_Q, W_pred, W_V):
    """Per-core input maps. Core c = batch (c//2), head-group (c%2)."""
    pk = _softmax(W_K.astype(np.float64)).astype(np.float32)
    pq = _softmax(W_Q.astype(np.float64)).astype(np.float32)
    pp = _softmax(W_pred.astype(np.float64)).astype(np.float32)
    pv = _softmax(W_V.astype(np.float64)).astype(np.float32)

    trit = np.triu(np.ones((CH, CH), np.float32))
    oneh = np.zeros((CH, NPR, NPR), np.float32)
    oneh[:, np.arange(NPR), np.arange(NPR)] = 1.0
    oneh = oneh.reshape(CH, NPR * NPR)
    strt = np.triu(np.ones((NPR, NPR), np.float32), k=1)
    eye8 = np.eye(NPR, dtype=np.float32)
    sel = np.zeros((NPR, NPR, CH), np.float32)
    sel[np.arange(NPR), np.arange(NPR), :] = 1.0
    sel = sel.reshape(NPR, NPR * CH)

    eye = np.eye(DH, dtype=np.float32)
    v_full = np.einsum("bpd,id->bpi", x_num, pv)  # [B, P, H] fp32, host-side

    in_maps = []
    for core in range(NCORES):
        b, hg = core // 2, core % 2
        heads = range(hg * HPC, (hg + 1) * HPC)
        W = np.zeros((DC, FH2), np.float32)
        for j, i in enumerate(heads):
            # ktilde cols: W[(v,g), j*64+h] = pk[i,v] * pp[i,h,g]
            W[:, j * DH : (j + 1) * DH] = (
                pk[i][:, None, None] * pp[i].T[None, :, :]
            ).reshape(DC, DH)
            # xq cols: W[(v,h), FH + j*64+h'] = pq[i,v] * delta(h,h')
            W[:, FH + j * DH : FH + (j + 1) * DH] = np.kron(pq[i][:, None], eye)
        # per-partition contiguous slice blocks
        xq8 = x_cat[b].T.reshape(KC, CH, 8, P // 8).transpose(2, 1, 0, 3)
        wq = W.reshape(KC, CH, FH2).transpose(1, 0, 2)
        # v in device layout [p, (chunk, head)]
        v_core = v_full[b][:, hg * HPC : (hg + 1) * HPC]  # [P, HPC]
        v_dev = np.ascontiguousarray(
            v_core.reshape(NCH, CH, HPC).transpose(1, 0, 2).reshape(CH, NCH * HPC)
        )
        in_maps.append(
            {
                "xct": np.ascontiguousarray(xq8).astype(_BF16),
                "w": np.ascontiguousarray(wq).astype(_BF16),
                "v": v_dev,
                "trit": trit.astype(_BF16),
                "oneh": oneh.astype(_BF16),
                "strt": strt.astype(_BF16),
                "eye8": eye8.astype(_BF16),
                "sel": sel.astype(_BF16),
            }
        )
    return in_maps


def _run(inputs, **spmd_kwargs):
    if "nc" not in _cache:
        _cache["nc"] = _build_program()
    nc = _cache["nc"]

    in_maps = _host_inputs(**inputs)
    res = run_bass_kernel_spmd(nc, in_maps, list(range(NCORES)), **spmd_kwargs)

    out = np.zeros((B, P, H), np.float32)
    for core in range(NCORES):
        b, hg = core // 2, core % 2
        z = res.results[core]["z"]  # [128, NCH*HPC]
        z = z.reshape(CH, NCH, HPC).transpose(1, 0, 2).reshape(P, HPC)
        out[b, :, hg * HPC : (hg + 1) * HPC] = z
    return out, res


def kernel(x_cat, x_num, W_K, W_Q, W_pred, W_V):
    out, _ = _run(
        dict(x_cat=x_cat, x_num=x_num, W_K=W_K, W_Q=W_Q, W_pred=W_pred, W_V=W_V)
    )
    return out


# revision 47
# speedup vs baseline: 1.1254x; 1.0959x over previous
"""Trainium2 Bass kernel for nn_NumAttention (sparse_attention).

Reference computation (per batch b, head i):
    k     = blockmix(x_cat, softmax(W_K)[i])            # [P, DH]
    xq    = blockmix(x_cat, softmax(W_Q)[i])            # [P, DH]
    q     = xq @ softmax(W_pred)[i]                     # [P, DH]
    v     = x_num @ softmax(W_V)[i]                     # [P]
    z[qp] = sum_{p<=qp} v[p] * (k[p] . q[qp])           # causal, no softmax

Key restructuring: attention here is softmax-free with scalar values, so it
is *linear*:  z[qp] = xq[qp] . S[qp]  with  S = cumsum_p(v[p] * ktilde[p,:])
where ktilde = k @ pp^T folds the W_pred mix into the k side.  The O(P^2)
score matrix is never materialized; per-core device work is one
[P,512]x[512,256] bf16 mix matmul (fp32 accumulate) plus a chunked cumsum.

The chunked cumsum: per 128-position chunk, S_c = triT_incl @ vk_c with the
inter-chunk carry folded in by *adding the exclusive block prefix Tex[c] to
vk_c's first row* - the inclusive triangular matmul then propagates it to
every row of the chunk.  This keeps pass 2 at one bf16 matmul per chunk
pair with a single stationary operand (no LDWEIGHTS churn, no carry
matmuls).  Block sums ride one accumulating PSUM tile; the 8-row prefix is
three tiny matmuls.

Sharding: 8 cores = 4 batches x 2 head-groups (4 heads each).  Host ships
x_cat[b] pre-transposed to feature-major bf16 (halves HBM traffic, no
on-device transposes), the tiny per-head effective weight matrices, and
host-computed v (x_num @ pv^T, 8 MFLOP).  A short burst of dummy matmuls
during the DMA head warms the PE HAM clock gate before the real mix stream.
"""

import numpy as np
import ml_dtypes

import concourse.bacc as bacc
import concourse.mybir as mybir
import concourse.tile as tile
from concourse.tile import add_dep_helper
from concourse.bass_utils import run_bass_kernel_spmd

B, P, DC, DN, H, DH = 4, 2048, 512, 64, 8, 64
NV = DC // DH
CH = 128          # positions per chunk
NCH = P // CH     # 16 chunks
NPR = NCH // 2    # 8 chunk pairs
HPC = 4           # heads per core
FH = HPC * DH     # 256 = stacked-head free width
FH2 = 2 * FH      # 512 = pair width
NCORES = 8
KC = DC // CH     # 4 feature K-chunks
NWARM = 7        # PE warm-up dummy matmuls

_BF16 = ml_dtypes.bfloat16

_cache = {}


def _softmax(x, axis=-1):
    e = np.exp(x - x.max(axis=axis, keepdims=True))
    return e / e.sum(axis=axis, keepdims=True)


def _build_program():
    nc = bacc.Bacc()
    f32 = mybir.dt.float32
    bf16 = mybir.dt.bfloat16
    mult = mybir.AluOpType.mult
    add = mybir.AluOpType.add

    # x_cat[b] transposed + eighth-blocked on host so each 256KB slice
    # DMA is per-partition contiguous (2KB descriptors)
    w_d = nc.dram_tensor("w", [CH, KC, FH2], bf16, kind="ExternalInput")
    xct_d = nc.dram_tensor("xct", [8, CH, KC, P // 8], bf16, kind="ExternalInput")
    # host-computed v in pos-chunk-major layout [p, (chunk, head)]
    v_d = nc.dram_tensor("v", [CH, NCH * HPC], f32, kind="ExternalInput")
    trit_d = nc.dram_tensor("trit", [CH, CH], bf16, kind="ExternalInput")
    # oneh[:, j*NPR + m] = (m == j): pair-j chunk-sum selector columns
    oneh_d = nc.dram_tensor("oneh", [CH, NPR * NPR], bf16, kind="ExternalInput")
    strt_d = nc.dram_tensor("strt", [NPR, NPR], bf16, kind="ExternalInput")
    eye8_d = nc.dram_tensor("eye8", [NPR, NPR], bf16, kind="ExternalInput")
    # sel[k, j*128+p] = (k == j): carry-broadcast selector
    sel_d = nc.dram_tensor("sel", [NPR, NPR * CH], bf16, kind="ExternalInput")
    z_d = nc.dram_tensor("z", [CH, NCH * HPC], f32, kind="ExternalOutput")

    with tile.TileContext(nc) as tc:
        with (
            tc.tile_pool(name="persist", bufs=1) as pers,
            tc.tile_pool(name="work", bufs=3) as work,
            tc.tile_pool(name="mixp", bufs=3, space="PSUM") as mixp,
            tc.tile_pool(name="sp", bufs=2, space="PSUM") as sp,
            tc.tile_pool(name="psmall", bufs=1, space="PSUM") as psmall,
        ):
            w_sb = pers.tile([CH, KC, FH2], bf16, tag="w_sb")
            xcT = pers.tile([CH, 8, KC, P // 8], bf16, tag="xcT")
            v_sb = pers.tile([CH, NCH * HPC], f32, tag="v_sb")
            trit_sb = pers.tile([CH, CH], bf16, tag="trit_sb")
            oneh_sb = pers.tile([CH, NPR * NPR], bf16, tag="oneh_sb")
            strt_sb = pers.tile([NPR, NPR], bf16, tag="strt_sb")
            eye8_sb = pers.tile([NPR, NPR], bf16, tag="eye8_sb")
            sel_sb = pers.tile([NPR, NPR * CH], bf16, tag="sel_sb")
            vk_sb = pers.tile([CH, NCH, FH], bf16, tag="vk_sb")
            q_sb = pers.tile([CH, NCH, FH], bf16, tag="q_sb")
            t2_sb = pers.tile([NPR, FH2], bf16, tag="t2_sb")
            tsum_sb = pers.tile([NPR, FH], bf16, tag="tsum_sb")
            texw_sb = pers.tile([NPR, FH2], bf16, tag="texw_sb")
            z_sb = pers.tile([CH, NCH * HPC], f32, tag="z_sb")
            dumw = pers.tile([CH, FH2], bf16, tag="dumw")

            # ---- PE warm-up: dummy matmuls on a memset tile release the HAM
            # clock throttle while the xcT DMA is still in flight
            nc.gpsimd.memset(dumw[:], 0.0)
            psum_dum = psmall.tile([CH, FH2], f32, tag="psum_dum")
            for i in range(NWARM):
                nc.tensor.matmul(
                    psum_dum[:], dumw[:, 0:CH], dumw[:], start=True, stop=True
                )

            # ---- loads (all plain HWDGE; x_cat pre-transposed on host).
            # The SDMA engines drain ALL outstanding transfers round-robin at
            # packet granularity, so an unordered burst makes the first xcT
            # slice (which gates the whole mix stream) finish last.  Stage the
            # transfers with explicit sync deps: {w, v, s0} first, then xcT
            # slice pairs in consumption order, constants last.
            # interleave slices across the two HWDGE rings (each ring
            # drains FIFO; the rings share the SDMA engines round-robin) so
            # arrival tracks consumption order: sync=[s0,s2,..], scalar=[w,s1,..]
            nc.sync.dma_start(out=xcT[:, 0], in_=xct_d[0])
            nc.scalar.dma_start(out=w_sb[:], in_=w_d[:])
            for s in range(2, 8, 2):
                nc.sync.dma_start(out=xcT[:, s], in_=xct_d[s])
            for s in range(1, 8, 2):
                nc.scalar.dma_start(out=xcT[:, s], in_=xct_d[s])
            nc.scalar.dma_start(out=v_sb[:], in_=v_d[:])
            nc.scalar.dma_start(out=trit_sb[:], in_=trit_d[:])
            nc.scalar.dma_start(out=oneh_sb[:], in_=oneh_d[:])
            nc.scalar.dma_start(out=strt_sb[:], in_=strt_d[:])
            nc.scalar.dma_start(out=eye8_sb[:], in_=eye8_d[:])
            nc.scalar.dma_start(out=sel_sb[:], in_=sel_d[:])

            # ---- pass 1: mix matmuls -> vk (bf16), q (bf16)
            for c in range(NCH):
                psum_mix = mixp.tile([CH, FH2], f32, tag="psum_mix")
                for kc in range(KC):
                    nc.tensor.matmul(
                        psum_mix[:],
                        xcT[:, c // 2, kc, (c % 2) * CH : (c % 2 + 1) * CH],
                        w_sb[:, kc, :],
                        start=(kc == 0),
                        stop=(kc == KC - 1),
                    )
                # vk[p, i, h] = ktilde[p, i, h] * v[p, i]
                nc.vector.tensor_tensor(
                    out=vk_sb[:, c, :].rearrange("p (i h) -> p i h", h=DH),
                    in0=psum_mix[:, 0:FH].rearrange("p (i h) -> p i h", h=DH),
                    in1=v_sb[:, c * HPC : (c + 1) * HPC].unsqueeze(2).broadcast_to(
                        [CH, HPC, DH]
                    ),
                    op=mult,
                )
                nc.scalar.copy(q_sb[:, c, :], psum_mix[:, FH:FH2])

            # ---- paired chunk sums (after the mix stream so the in-order PE
            # never stalls on DVE vk completion mid-mix)
            psum_T2 = psmall.tile([NPR, FH2], f32, tag="psum_T2")
            for j in range(NPR):
                # T2[j] = [colsum(vk_{2j}) | colsum(vk_{2j+1})]
                nc.tensor.matmul(
                    psum_T2[:],
                    oneh_sb[:, j * NPR : (j + 1) * NPR],
                    vk_sb[:, 2 * j : 2 * j + 2, :].rearrange("p c f -> p (c f)"),
                    start=(j == 0),
                    stop=(j == NPR - 1),
                )

            # ---- block prefix on pair sums.  Dummy matmuls keep the PE (and
            # its HAM clock) busy while the short DVE/ACT prefix chain runs.
            for i in range(4):
                nc.tensor.matmul(
                    psum_dum[:], dumw[:, 0:CH], dumw[:], start=True, stop=True
                )
            nc.vector.tensor_copy(t2_sb[:], psum_T2[:])
            nc.vector.tensor_tensor(
                out=tsum_sb[:], in0=t2_sb[:, 0:FH], in1=t2_sb[:, FH:FH2], op=add
            )
            psum_texw = psmall.tile([NPR, FH2], f32, tag="psum_texw")
            # left half: Tex[2j]   = sum_{j'<j} Tsum[j']
            nc.tensor.matmul(
                psum_texw[:, 0:FH], strt_sb[:], tsum_sb[:], start=True, stop=True
            )
            # right half: Tex[2j+1] = Tex[2j] + T[2j]
            nc.tensor.matmul(
                psum_texw[:, FH:FH2], strt_sb[:], tsum_sb[:], start=True, stop=False
            )
            nc.tensor.matmul(
                psum_texw[:, FH:FH2], eye8_sb[:], t2_sb[:, 0:FH], start=False, stop=True
            )
            nc.vector.tensor_copy(texw_sb[:], psum_texw[:])
            for i in range(6):
                nc.tensor.matmul(
                    psum_dum[:], dumw[:, 0:CH], dumw[:], start=True, stop=True
                )

            # ---- pass 2 (paired): S = triT @ vk + carry;  z = rowsum(q * S)
            for j in range(NPR):
                psum_S = sp.tile([CH, FH2], f32, tag="psum_S")
                nc.tensor.matmul(
                    psum_S[:],
                    trit_sb[:],
                    vk_sb[:, 2 * j : 2 * j + 2, :].rearrange("p c f -> p (c f)"),
                    start=True,
                    stop=False,
                )
                nc.tensor.matmul(
                    psum_S[:],
                    sel_sb[:, j * CH : (j + 1) * CH],
                    texw_sb[:],
                    start=False,
                    stop=True,
                )
                # drain S to SBUF on the otherwise-idle ACT engine so both
                # DVE ops below run on 16-bit SBUF operands (fast path)
                s_sb = work.tile([CH, FH2], bf16, tag="s_sb")
                nc.scalar.copy(s_sb[:], psum_S[:])
                prod = work.tile([CH, FH2], bf16, tag="prod")
                nc.vector.tensor_tensor(
                    out=prod[:],
                    in0=q_sb[:, 2 * j : 2 * j + 2, :].rearrange("p c f -> p (c f)"),
                    in1=s_sb[:],
                    op=mult,
                )
                nc.vector.tensor_reduce(
                    out=z_sb[:, 2 * j * HPC : (2 * j + 2) * HPC].rearrange(
                        "p (c i) -> p c i", i=HPC
                    ),
                    in_=prod[:].rearrange("p (c i h) -> p c i h", i=HPC, h=DH),
                    axis=mybir.AxisListType.X,
                    op=add,
                )

            nc.sync.dma_start(out=z_d[:], in_=z_sb[:])

    nc.finalize()
    return nc


def _host_inputs(x_cat, x_num, W_K, W_Q, W_pred, W_V):
    """Per-core input maps. Core c = batch (c//2), head-group (c%2)."""
    pk = _softmax(W_K.astype(np.float64)).astype(np.float32)
    pq = _softmax(W_Q.astype(np.float64)).astype(np.float32)
    pp = _softmax(W_pred.astype(np.float64)).astype(np.float32)
    pv = _softmax(W_V.astype(np.float64)).astype(np.float32)

    trit = np.triu(np.ones((CH, CH), np.float32))
    oneh = np.zeros((CH, NPR, NPR), np.float32)
    oneh[:, np.arange(NPR), np.arange(NPR)] = 1.0
    oneh = oneh.reshape(CH, NPR * NPR)
    strt = np.triu(np.ones((NPR, NPR), np.float32), k=1)
    eye8 = np.eye(NPR, dtype=np.float32)
    sel = np.zeros((NPR, NPR, CH), np.float32)
    sel[np.arange(NPR), np.arange(NPR), :] = 1.0
    sel = sel.reshape(NPR, NPR * CH)

    eye = np.eye(DH, dtype=np.float32)
    v_full = np.einsum("bpd,id->bpi", x_num, pv)  # [B, P, H] fp32, host-side

    in_maps = []
    for core in range(NCORES):
        b, hg = core // 2, core % 2
        heads = range(hg * HPC, (hg + 1) * HPC)
        W = np.zeros((DC, FH2), np.float32)
        for j, i in enumerate(heads):
            # ktilde cols: W[(v,g), j*64+h] = pk[i,v] * pp[i,h,g]
            W[:, j * DH : (j + 1) * DH] = (
                pk[i][:, None, None] * pp[i].T[None, :, :]
            ).reshape(DC, DH)
            # xq cols: W[(v,h), FH + j*64+h'] = pq[i,v] * delta(h,h')
            W[:, FH + j * DH : FH + (j + 1) * DH] = np.kron(pq[i][:, None], eye)
        # per-partition contiguous slice blocks
        xq8 = x_cat[b].T.reshape(KC, CH, 8, P // 8).transpose(2, 1, 0, 3)
        wq = W.reshape(KC, CH, FH2).transpose(1, 0, 2)
        # v in device layout [p, (chunk, head)]
        v_core = v_full[b][:, hg * HPC : (hg + 1) * HPC]  # [P, HPC]
        v_dev = np.ascontiguousarray(
            v_core.reshape(NCH, CH, HPC).transpose(1, 0, 2).reshape(CH, NCH * HPC)
        )
        in_maps.append(
            {
                "xct": np.ascontiguousarray(xq8).astype(_BF16),
                "w": np.ascontiguousarray(wq).astype(_BF16),
                "v": v_dev,
                "trit": trit.astype(_BF16),
                "oneh": oneh.astype(_BF16),
                "strt": strt.astype(_BF16),
                "eye8": eye8.astype(_BF16),
                "sel": sel.astype(_BF16),
            }
        )
    return in_maps


def _run(inputs, **spmd_kwargs):
    if "nc" not in _cache:
        _cache["nc"] = _build_program()
    nc = _cache["nc"]

    in_maps = _host_inputs(**inputs)
    res = run_bass_kernel_spmd(nc, in_maps, list(range(NCORES)), **spmd_kwargs)

    out = np.zeros((B, P, H), np.float32)
    for core in range(NCORES):
        b, hg = core // 2, core % 2
        z = res.results[core]["z"]  # [128, NCH*HPC]
        z = z.reshape(CH, NCH, HPC).transpose(1, 0, 2).reshape(P, HPC)
        out[b, :, hg * HPC : (hg + 1) * HPC] = z
    return out, res


def kernel(x_cat, x_num, W_K, W_Q, W_pred, W_V):
    out, _ = _run(
        dict(x_cat=x_cat, x_num=x_num, W_K=W_K, W_Q=W_Q, W_pred=W_pred, W_V=W_V)
    )
    return out
